# revision 44
# baseline (speedup 1.0000x reference)
"""LIF spiking-neuron recurrence kernel for Trainium2 (Bass/Tile, 8-core SPMD).

Problem: x [32, 128, 32, 32, 8] f32, time on the LAST axis (T=8).
    u_0 = x_0;  o_t = (u_t > Vth);  u_{t+1} = TAU * u_t * (1 - o_t) + x_{t+1}
Output: spikes o [32, 128, 32, 32, 8] f32 (0.0 / 1.0).

Sharding: pure data-parallel over the batch dim (32 -> 4 per core, 8 cores),
no communication. The host lays each core's shard out t-plane-major
([pixels, T] -> [T, pixels] per 1024-pixel row group) so every on-chip
operand is dense unit-stride.

State is held in fp16 (x planes converted on the host): the DVE computes
fp32-internally with an exact fp32 0.2 threshold, so only the fp16
storage rounding of u/x perturbs results — measured 589/33.5M flipped
spikes (rel err 6.8e-3) on the fixed harness input, 3x inside the 2e-2
gate, and deterministic. fp16 also halves input HBM traffic.

Per-core schedule (2 row-tiles x 8 t-planes of [128, 2048] fp16):
  DVE   mt = (u_t is_le Vth) mult TAU         tensor_scalar, fp16 4x mode
        w = mt * u_t                          tensor_tensor, fp16 2x mode
        u_{t+1} = w + x_{t+1}                 tensor_tensor, fp16 2x mode
        (three packed-fp16 ops beat the two fused scalar_tensor_tensor
         ops used for f32 — stt has no fp16 perf-mode uops)
  ACT   s_t = Sign(-u_t/Vth + 1) -> bf16      inverted spike sign per step
  PE    q += (2^t I)^T . s_t                  diagonal matmul accumulates the
                                              sign planes into PSUM f32
  ACT   out_u8 = 0.5*q + 127.5                = sum_t (1-o_t) 2^t; host
                                              decodes spikes from 255 - byte
The kernel stores ONE byte per pixel (bit t = spike at step t) instead of
8 f32 words: output HBM traffic drops 32x. The mask-mult and x0.25 scale
are exact in fp16 (mask is {0,1}-valued; TAU is a power of two).
"""

import numpy as np
import ml_dtypes

import bass_rust
import concourse.bass as bass
import concourse.mybir as mybir
import concourse.tile as tile
from concourse.bass_utils import run_bass_kernel_spmd

VTH = 0.2
TAU = 0.25

N_CORES = 8
FULL_SHAPE = (32, 128, 32, 32, 8)
B_PER_CORE = FULL_SHAPE[0] // N_CORES  # 4
T = FULL_SHAPE[-1]  # 8

ROWS = 128  # per-core partition rows: 4*128*32*32*8 / FREE
FREE = 32768  # free dim per row (T * C)
C = FREE // T  # 4096 pixels per partition row (both old row-tiles side
#               by side: at fp16 op durations the per-op sem/dispatch
#               overhead dominates, so fewer, wider ops win)
N_TILES = 1
NCH = C // 512  # 8 matmul chunks per plane

_cache: dict = {}


def _split_multi_waits(nc: bass.Bass) -> int:
    """Hoist all-but-one embedded sync waits onto standalone EventSemaphore
    instructions. The walrus build behind bass2jax rejects >1 sync wait per
    instruction ("Too many sync wait commands"); a standalone wait on the
    same engine stream immediately before is semantically identical."""
    n = 0
    for fn in nc.m.functions:
        for block in fn.blocks:
            out = []
            changed = False
            for ins in block.instructions:
                si = ins.sync_info
                waits = list(si.on_wait) if si is not None else []
                if len(waits) > 1:
                    for k, w in enumerate(waits[:-1]):
                        ev = mybir.InstEventSemaphore(
                            name=f"{ins.name}-hw{k}", ins=[], outs=[]
                        )
                        ev.sync_info = bass_rust.SyncInfo(
                            on_wait=[w], on_update=[]
                        )
                        ev.engine = ins.engine
                        nc.inst_map[ev.name] = ev
                        out.append(ev)
                        n += 1
                    si.on_wait = [waits[-1]]
                    changed = True
                out.append(ins)
            if changed:
                block.instructions = out
    return n


def _build_bass() -> bass.Bass:
    f32 = mybir.dt.float32
    f16 = mybir.dt.float16
    bf16 = mybir.dt.bfloat16
    f8 = mybir.dt.float8e4
    u8 = mybir.dt.uint8
    Alu = mybir.AluOpType
    Act = mybir.ActivationFunctionType

    nc = bass.Bass(trn_type="TRN2")

    x_d = nc.dram_tensor("x", [ROWS, FREE], f16, kind="ExternalInput")
    w_d = nc.dram_tensor("w", [128, T * 128], f8, kind="ExternalInput")
    y_d = nc.dram_tensor("y", [ROWS, C], u8, kind="ExternalOutput")

    with tile.TileContext(nc) as tc:
        with (
            tc.tile_pool(name="pin", bufs=8) as pin,
            tc.tile_pool(name="pu", bufs=3) as pu,
            tc.tile_pool(name="pv", bufs=2) as pv,
            tc.tile_pool(name="ps", bufs=4) as ps,
            tc.tile_pool(name="po", bufs=2) as po,
            tc.tile_pool(name="pw", bufs=1) as pw,
            tc.psum_pool(name="pq", bufs=1) as pq,
        ):
            w = pw.tile([128, T * 128], f8, tag="w")

            # ---- all loads, in latency-critical order ----
            # tile0 t0/t1 as quarter-planes (head pipeline starts on the
            # first quarter), tile1's first plane slotted mid-tile0 so it
            # lands long before tile0's compute finishes, bulk last.
            xp0 = [pin.tile([128, C], f16, tag="xp", name=f"xp0_{t}") for t in range(T)]
            # all x loads on SP's queue: a second HWDGE ring (ACT/GpSimd)
            # steals SDMA-engine packets from SP's ring and slows the head
            # fp16 halves transfer times, so the head cascade is
            # issue-rate-bound: 1024-col chunks halve the issue count
            for t in (0, 1):
                for ch in range(C // 1024):
                    nc.sync.dma_start(
                        xp0[t][:, ch * 1024 : (ch + 1) * 1024],
                        x_d[0:128, t * C + ch * 1024 : t * C + (ch + 1) * 1024],
                    )
            # weight load rides ACT's queue; the first matmul needs w
            # only ~13us in
            nc.scalar.dma_start(w, w_d[:, :])

            def load0(t):
                nc.sync.dma_start(xp0[t], x_d[0:128, t * C : (t + 1) * C])

            for t in range(2, T):
                load0(t)

            for i in range(N_TILES):
                rows = slice(i * 128, (i + 1) * 128)
                xp = xp0

                q = pq.tile([128, C], f32, tag="q")
                u_cur = xp[0]
                for t in range(T):
                    head = i == 0 and t == 0
                    tail = t == T - 1
                    # spike sign plane: s = Sign(u_t - Vth) in {-1, +1};
                    # chunked on the last plane so sign/pack/convert/store
                    # pipeline instead of serializing the kernel tail
                    s = ps.tile([128, C], f8, tag="s")
                    for ch in (
                        range(4) if (tail and i == N_TILES - 1) else (None,)
                    ):
                        cols = slice(0, C) if ch is None else slice(
                            ch * 1024, (ch + 1) * 1024
                        )
                        # s = Sign(-u/Vth + 1) = -Sign(u - Vth): uses the
                        # pre-registered 1.0 const AP, so no extra memset
                        # delays the entry barrier. The FMA is exactly
                        # rounded, so the sign matches -sign(u - Vth)
                        # everywhere but a ~1e-39 sliver. Host inverts.
                        nc.scalar.activation(
                            s[:, cols], u_cur[:, cols], Act.Sign,
                            bias=1.0, scale=-1.0 / VTH,
                        )
                    # pack into PSUM: q[:, ch] += (2^t I)^T . s[:, ch]
                    for ch in range(NCH):
                        cols = slice(ch * 512, (ch + 1) * 512)
                        nc.tensor.matmul(
                            q[:, cols],
                            w[:, t * 128 : (t + 1) * 128],
                            s[:, cols],
                            start=(t == 0),
                            stop=tail,
                        )
                    if not tail:
                        # v = (u <= Vth) * u ; u' = TAU*v + x_{t+1}
                        # (chunked at the head to start on the first
                        # quarter-plane load, and on t=6 to feed the
                        # chunked tail plane early)
                        v = pv.tile([128, C], f16, tag="v")
                        mt = pv.tile([128, C], f16, tag="mt")
                        u_nxt = pu.tile([128, C], f16, tag="u")
                        if head:
                            for ch in range(C // 1024):
                                cols = slice(ch * 1024, (ch + 1) * 1024)
                                nc.vector.tensor_scalar(
                                    mt[:, cols], u_cur[:, cols], VTH, TAU,
                                    Alu.is_le, Alu.mult,
                                )
                                nc.vector.tensor_tensor(
                                    v[:, cols], mt[:, cols], u_cur[:, cols],
                                    Alu.mult,
                                )
                                nc.vector.tensor_tensor(
                                    u_nxt[:, cols], v[:, cols],
                                    xp[t + 1][:, cols], Alu.add,
                                )
                        else:
                            # fp16 3-op form: tensor_scalar has fp16 packed
                            # perf-mode uops and tensor_tensor has 2x_1p;
                            # scalar_tensor_tensor is 1x-only, so the fused
                            # form wins for f32 but loses for fp16.
                            nc.vector.tensor_scalar(
                                mt, u_cur, VTH, TAU, Alu.is_le, Alu.mult
                            )
                            nc.vector.tensor_tensor(v, mt, u_cur, Alu.mult)
                            if t == T - 2 and i == N_TILES - 1:
                                # chunked so the tail plane's sign/pack/store
                                # pipeline starts on the first quarter
                                for ch in range(4):
                                    cols = slice(ch * 1024, (ch + 1) * 1024)
                                    nc.vector.tensor_tensor(
                                        u_nxt[:, cols], v[:, cols],
                                        xp[t + 1][:, cols], Alu.add,
                                    )
                            else:
                                nc.vector.tensor_tensor(
                                    u_nxt, v, xp[t + 1], Alu.add
                                )
                        u_cur = u_nxt

                # packed byte: (q + 255) / 2 = sum_t o_t 2^t, exact;
                # chunked so each 512-col group converts + stores as soon as
                # its accumulation group closes. The last tile converts on
                # DVE (idle by then; ACT's serial queue was the tail path).
                qu8 = po.tile([128, C], u8, tag="qu8")
                for ch in range(4):
                    cols = slice(ch * 1024, (ch + 1) * 1024)
                    if i == N_TILES - 1 and ch < 2:
                        # tail split: DVE (idle right after the last stt)
                        # converts the first two chunks while ACT is still
                        # finishing the t7 Sign chunks; ACT takes the rest.
                        # All tail stores go to SP, whose queue is empty —
                        # store issues on ACT's queue would serialize with
                        # its remaining ACTIVATEs.
                        nc.vector.tensor_scalar(
                            qu8[:, cols], q[:, cols], 0.5, 127.5,
                            Alu.mult, Alu.add,
                        )
                    else:
                        nc.scalar.activation(
                            qu8[:, cols], q[:, cols], Act.Copy,
                            bias=127.5, scale=0.5,
                        )
                    nc.sync.dma_start(y_d[rows, cols], qu8[:, cols])

    _split_multi_waits(nc)
    return nc


def _shard(x: np.ndarray, c: int) -> np.ndarray:
    """Core c's shard, t-plane-major: [ROWS, C, T] -> [ROWS, T, C] -> flat."""
    s = x[c * B_PER_CORE : (c + 1) * B_PER_CORE].reshape(2, ROWS, C // 2, T)
    s = s.transpose(1, 3, 0, 2).reshape(ROWS, T, C)  # [rows, T, 4096]
    return np.ascontiguousarray(s).astype(np.float16).reshape(ROWS, FREE)


def _weights() -> np.ndarray:
    # fp8-e4m3 represents +-1 and 2^t (t<8) exactly; fp8 matmuls run at
    # 2x the bf16 rate on PE
    w = np.zeros((128, T * 128), dtype=ml_dtypes.float8_e4m3)
    for t in range(T):
        w[:, t * 128 : (t + 1) * 128] = (2.0**t) * np.eye(
            128, dtype=ml_dtypes.float8_e4m3
        )
    return w


def _in_maps(x: np.ndarray) -> list[dict]:
    w = _weights()
    return [{"x": _shard(x, c), "w": w} for c in range(N_CORES)]


def _unshard(y: np.ndarray) -> np.ndarray:
    """Decode one core's packed-byte output [ROWS, C] u8 -> spikes f32.
    The device packs inverted sign digits (see the Sign call), so the
    spike byte is 255 - y."""
    y = np.uint8(255) - y
    bits = (y[:, :, None] >> np.arange(T, dtype=np.uint8)) & np.uint8(1)
    b = bits.reshape(ROWS, 2, C // 2, T).transpose(1, 0, 2, 3)
    return b.astype(np.float32).reshape(B_PER_CORE, *FULL_SHAPE[1:])


def kernel(x: np.ndarray) -> np.ndarray:
    assert x.shape == FULL_SHAPE, x.shape
    in_dtype = x.dtype

    if "nc" not in _cache:
        _cache["nc"] = _build_bass()
    nc = _cache["nc"]

    x = np.ascontiguousarray(x, dtype=np.float32)
    res = run_bass_kernel_spmd(nc, _in_maps(x), core_ids=list(range(N_CORES)))
    out = np.concatenate(
        [_unshard(res.results[c]["y"]) for c in range(N_CORES)], axis=0
    )
    return out.astype(in_dtype, copy=False)
